# revision 40
# baseline (speedup 1.0000x reference)
"""MoE FFN (top-1 routing) Trainium2 kernel — expert-parallel across 8 cores.

Strategy (per the expert-parallel sharding hint): the router gate and the
token dispatch ARE the sharding step, performed on the host inside kernel():
  - host computes router logits (x @ Wg + bg) and argmax expert ids
  - tokens are gathered per expert, padded to capacity C = max expert load
  - core e receives expert e's W1/W2/b1 plus its routed tokens, pre-tiled
    into contiguous-DMA layouts
  - the device runs the full FFN (both matmuls + exact gelu) in float32r
  - host scatters per-expert outputs back (adds b2 there, it is per-token
    constant) and un-shards to the full [B, S, D] output

Device kernel per core (C tokens, D=1024, H=4096):
  phase A: hT[m*128+j, c] = gelu(sum_k W1tile[k,m].T x^T[k] + b1), m in 0..31
  phase B: yT[d*128+j, c] = sum_k W2tile[k,d].T hT[k],             d in 0..7
Both matmuls consume the weights in their natural [K, M] orientation as the
stationary operand, so no transposes are needed anywhere on the device.

DMA ring assignment: w1 stream on the sync HWDGE ring; xt + w2 stream on the
scalar HWDGE ring; output writes on gpsimd SWDGE.  A short burst of scratch
matmuls at kernel start keeps the PE HAM clock-gate warm through the input
load prologue.
"""

import os
import sys

import numpy as np

for _p in ("/opt/trn_rl_repo", "/root/.axon_site/_ro/trn_rl_repo"):
    if os.path.isdir(_p) and _p not in sys.path:
        sys.path.insert(0, _p)

D_MODEL = 1024
D_HIDDEN = 4096
N_EXPERTS = 8
N_CORES = 8
P = 128
KD = D_MODEL // P  # 8 k-chunks over d_model
MH = D_HIDDEN // P  # 32 m-chunks over d_hidden
MG = 2  # m-chunks per w1 DMA (1 MiB transfers)
N_WARM = 7  # scratch f32 matmuls (~1.1us each) to warm the PE clock gate

_compiled_cache = {}

# Set by the most recent kernel() call when BASS_KERNEL_TRACE=1: HW exec ns.
last_exec_time_ns = None
last_results = None


def _chunk_sizes(C):
    """Split C token columns into chunks <= 512, as evenly as possible.

    C >= 512 always (max expert load >= 4096/8), so chunks land in
    [256, 512] and float32r matmuls run at full 1 cycle/row speed.
    """
    nch = -(-C // 512)
    base, rem = divmod(C, nch)
    return [base + 1] * rem + [base] * (nch - rem)


def _w1_group_sizes():
    """m-chunks per w1 DMA group: two small leading groups so the first
    weight tile lands quickly at kernel start, then 1 MiB groups."""
    sizes = [1, 1]
    rest = MH - sum(sizes)
    sizes += [MG] * (rest // MG)
    return sizes


def _build_program(C):
    import concourse.mybir as mybir
    import concourse.tile as tile
    from concourse import bacc

    f32 = mybir.dt.float32
    f32r = mybir.dt.float32r

    nc = bacc.Bacc("TRN2", target_bir_lowering=False, debug=False,
                   num_devices=N_CORES)

    # Host-pretiled inputs (layouts chosen so each DMA is contiguous):
    #   xt  [128, KD*C] chunk-major: for token-chunk j (offset pf_j, len ch_j)
    #         xt[p, pf_j*KD + k*ch_j + c] = x[pf_j + c, k*128+p]
    #   w1  [128, MH*KD*128] grouped: group (m-offset o, size s) occupies
    #         cols [o*KD*P, (o+s)*KD*P); w1[p, off + (i*KD+k)*128+j] =
    #         W1[k*128+p, (o+i)*128+j]
    #   w2  [KD, 128, MH*128]       w2[d, p, k*128+j] = W2[k*128+p, d*128+j]
    #   b1t [128, MH]               b1t[p, m] = b1[m*128+p]
    # Output:
    #   yt  [KD, 128, C]            yt[d, p, c] = y[c, d*128+p]  (pre-b2)
    xt_d = nc.declare_dram_parameter("xt", [P, KD * C], f32r, isOutput=False)
    w1_d = nc.declare_dram_parameter("w1", [P, MH * KD * P], f32r, isOutput=False)
    w2_d = nc.declare_dram_parameter("w2", [KD, P, MH * P], f32r, isOutput=False)
    b1_d = nc.declare_dram_parameter("b1t", [P, MH], f32, isOutput=False)
    yt_d = nc.declare_dram_parameter("yt", [KD, P, C], f32, isOutput=True)

    chunks = _chunk_sizes(C)
    prefix = [0]
    for cn in chunks:
        prefix.append(prefix[-1] + cn)
    gsizes = _w1_group_sizes()

    with tile.TileContext(nc) as tc:
        with (
            tc.tile_pool(name="persist", bufs=1) as persist,
            tc.tile_pool(name="w1p", bufs=5) as w1p,
            tc.tile_pool(name="w2p", bufs=3) as w2p,
            tc.tile_pool(name="outp", bufs=4) as outp,
            tc.tile_pool(name="psum", bufs=6, space="PSUM") as psum,
        ):
            # --- PE warm-up: keep HAM at 8/8 through the input-load prologue
            scratch = persist.tile([P, 512], f32)
            nc.vector.memset(scratch[:], 0.0)
            warm_ps = psum.tile([P, 512], mybir.dt.float32, tag="warm", bufs=1)
            for _ in range(N_WARM):
                nc.tensor.matmul(warm_ps[:], scratch[:, :P], scratch[:],
                                 start=True, stop=True)

            # --- input loads: xt token-chunk blocks lead both HWDGE rings;
            # the first chunk (all the first matmul groups need) is the
            # smaller critical mass and rides the sync ring.
            xt = persist.tile([P, KD * C], f32r)
            split = prefix[1] * KD
            nc.sync.dma_start(out=xt[:, :split], in_=xt_d[:, :split])
            nc.scalar.dma_start(out=xt[:, split:], in_=xt_d[:, split:])
            b1t = persist.tile([P, MH], f32)
            nc.gpsimd.dma_start(out=b1t[:], in_=b1_d[:])
            ht = persist.tile([P, MH * C], f32r)

            # w2 tiles are prefetched on the scalar ring: the first bufs-many
            # late in phase A (so they don't contend with the w1 stream),
            # the rest at prefetch distance 3 inside phase B.
            w2_tiles = [None] * KD

            def load_w2(d):
                t = w2p.tile([P, MH * P], f32r, tag="w2d")
                eng = nc.scalar if d % 2 == 0 else nc.sync
                eng.dma_start(out=t[:], in_=w2_d[d])
                w2_tiles[d] = t

            # ---- Phase A: hT = gelu(W1^T x^T + b1) ----
            m0 = 0
            for g, gs in enumerate(gsizes):
                w1g = w1p.tile([P, MG * KD * P], f32r, tag="w1g")
                w1_eng = nc.sync if g % 2 == 0 else nc.scalar
                w1_eng.dma_start(
                    out=w1g[:, :gs * KD * P],
                    in_=w1_d[:, m0 * KD * P:(m0 + gs) * KD * P])
                if g == len(gsizes) - 2:
                    load_w2(0)
                elif g == len(gsizes) - 1:
                    load_w2(1)
                for i in range(gs):
                    m = m0 + i
                    for j, cn in enumerate(chunks):
                        ps = psum.tile([P, 512], mybir.dt.float32, tag="ps")
                        xoff = prefix[j] * KD
                        for k in range(KD):
                            nc.tensor.matmul(
                                ps[:, :cn],
                                w1g[:, (i * KD + k) * P:(i * KD + k + 1) * P],
                                xt[:, xoff + k * cn:xoff + (k + 1) * cn],
                                start=(k == 0),
                                stop=(k == KD - 1),
                            )
                        nc.scalar.activation(
                            ht[:, m * C + prefix[j]:m * C + prefix[j] + cn],
                            ps[:, :cn],
                            mybir.ActivationFunctionType.Gelu,
                            bias=b1t[:, m:m + 1],
                        )
                m0 += gs

            # ---- Phase B: yT = W2^T hT ----
            for d in range(KD):
                w2d = w2_tiles[d]
                for j, cn in enumerate(chunks):
                    c0 = prefix[j]
                    ps = psum.tile([P, 512], mybir.dt.float32, tag="ps")
                    for k in range(MH):
                        nc.tensor.matmul(
                            ps[:, :cn],
                            w2d[:, k * P:(k + 1) * P],
                            ht[:, k * C + c0:k * C + c0 + cn],
                            start=(k == 0),
                            stop=(k == MH - 1),
                        )
                    ot = outp.tile([P, 512], f32, tag="ot")
                    nc.vector.tensor_copy(ot[:, :cn], ps[:, :cn])
                    nc.sync.dma_start(
                        out=yt_d[d, :, c0:c0 + cn], in_=ot[:, :cn]
                    )
                if d + 2 < KD:
                    load_w2(d + 2)

    nc.compile()
    return nc


def _get_program(C):
    if C not in _compiled_cache:
        _compiled_cache[C] = _build_program(C)
    return _compiled_cache[C]


def kernel(x, Wg, bg, W1, b1, W2, b2):
    global last_exec_time_ns, last_results
    from concourse.bass_utils import run_bass_kernel_spmd

    x = np.asarray(x, dtype=np.float32)
    Wg = np.asarray(Wg, dtype=np.float32)
    bg = np.asarray(bg, dtype=np.float32)
    W1 = np.asarray(W1, dtype=np.float32)
    b1 = np.asarray(b1, dtype=np.float32)
    W2 = np.asarray(W2, dtype=np.float32)
    b2 = np.asarray(b2, dtype=np.float32)

    B, S, D = x.shape
    T = B * S
    xf = x.reshape(T, D)

    # ---- Router (replicated gate, computed host-side as the dispatch step)
    logits = xf @ Wg + bg
    eidx = np.argmax(logits, axis=-1)

    tok = [np.nonzero(eidx == e)[0] for e in range(N_EXPERTS)]
    counts = [len(t) for t in tok]
    Cfull = max(max(counts), 512)
    Cfull = ((Cfull + 7) // 8) * 8  # mild alignment for DMA friendliness

    # SBUF fits a single pass up to ~C=704.  For pathologically imbalanced
    # routing, run multiple token blocks through the same program (weights
    # are re-sent per block; correctness over speed in that corner case).
    C_MAX = 704
    C = min(Cfull, C_MAX)
    n_blocks = -(-Cfull // C) if Cfull > C_MAX else 1
    if n_blocks > 1:
        C = min(C_MAX, ((-(-Cfull // n_blocks) + 7) // 8) * 8)

    nc = _get_program(C)
    chunks = _chunk_sizes(C)
    prefix = [0]
    for cn in chunks:
        prefix.append(prefix[-1] + cn)

    # ---- Per-core static operands (weights/biases, block-independent)
    w1s, w2s, b1s = [], [], []
    gsizes = _w1_group_sizes()
    for e in range(N_EXPERTS):
        # w1 flat [P, MH*KD*P]: per m-chunk block (i*KD+k)*P+j packing,
        # matching the device's per-group column slabs for any grouping.
        w1s.append(np.ascontiguousarray(
            W1[e].reshape(KD, P, MH, P)
            .transpose(2, 1, 0, 3)   # [m, p, k, j]
            .transpose(1, 0, 2, 3)   # [p, m, k, j]
            .reshape(P, MH * KD * P)
        ))
        w2s.append(np.ascontiguousarray(
            W2[e].reshape(MH, P, KD, P).transpose(2, 1, 0, 3).reshape(KD, P, MH * P)
        ))
        b1s.append(np.ascontiguousarray(b1[e].reshape(MH, P).T))

    trace = os.environ.get("BASS_KERNEL_TRACE", "") == "1"
    if trace:
        try:
            import axon_profile_shim

            axon_profile_shim.install()
        except ImportError:
            pass

    out = np.zeros((T, D), dtype=np.float32)
    for blk in range(n_blocks):
        in_maps = []
        blk_tok = []
        for e in range(N_EXPERTS):
            te = tok[e][blk * C:(blk + 1) * C]
            blk_tok.append(te)
            n_e = len(te)
            xe = xf[te]  # [n_e, D]
            # chunk-major xt: block j holds [P, KD*ch_j]
            xt = np.zeros((P, KD * C), dtype=np.float32)
            xeT = xe.T.reshape(KD, P, n_e)
            for j, cn in enumerate(chunks):
                lo = prefix[j]
                if lo >= n_e:
                    break
                hi = min(lo + cn, n_e)
                for k in range(KD):
                    xt[:, lo * KD + k * cn:lo * KD + k * cn + (hi - lo)] = \
                        xeT[k, :, lo:hi]
            in_maps.append({"xt": xt, "w1": w1s[e], "w2": w2s[e],
                            "b1t": b1s[e]})

        res = run_bass_kernel_spmd(nc, in_maps, list(range(N_CORES)),
                                   trace=trace)
        last_exec_time_ns = res.exec_time_ns
        last_results = res

        # ---- Combine: scatter tokens back, add b2 host-side
        for e in range(N_EXPERTS):
            te = blk_tok[e]
            n_e = len(te)
            if n_e == 0:
                continue
            yt = res.results[e]["yt"]  # [KD, P, C]
            ye = yt.reshape(D, C)[:, :n_e].T  # [n_e, D]
            out[te] = ye + b2[e][None, :]
    return out.reshape(B, S, D)


# revision 43
# speedup vs baseline: 1.0686x; 1.0686x over previous
"""MoE FFN (top-1 routing) Trainium2 kernel — expert-parallel across 8 cores.

Strategy (per the expert-parallel sharding hint): the router gate and the
token dispatch ARE the sharding step, performed on the host inside kernel():
  - host computes router logits (x @ Wg + bg) and argmax expert ids
  - tokens are gathered per expert, padded to capacity C = max expert load
  - core e receives expert e's W1/W2/b1 plus its routed tokens, pre-tiled
    into contiguous-DMA layouts
  - the device runs the full FFN (both matmuls + exact gelu) in float32r
  - host scatters per-expert outputs back (adds b2 there, it is per-token
    constant) and un-shards to the full [B, S, D] output

Device kernel per core (C tokens, D=1024, H=4096):
  phase A: hT[m*128+j, c] = gelu(sum_k W1tile[k,m].T x^T[k] + b1), m in 0..31
  phase B: yT[d*128+j, c] = sum_k W2tile[k,d].T hT[k],             d in 0..7
Both matmuls consume the weights in their natural [K, M] orientation as the
stationary operand, so no transposes are needed anywhere on the device.

DMA ring assignment: w1 stream on the sync HWDGE ring; xt + w2 stream on the
scalar HWDGE ring; output writes on gpsimd SWDGE.  A short burst of scratch
matmuls at kernel start keeps the PE HAM clock-gate warm through the input
load prologue.
"""

import os
import sys

import numpy as np

for _p in ("/opt/trn_rl_repo", "/root/.axon_site/_ro/trn_rl_repo"):
    if os.path.isdir(_p) and _p not in sys.path:
        sys.path.insert(0, _p)

D_MODEL = 1024
D_HIDDEN = 4096
N_EXPERTS = 8
N_CORES = 8
P = 128
KD = D_MODEL // P  # 8 k-chunks over d_model
MH = D_HIDDEN // P  # 32 m-chunks over d_hidden
MG = 2  # m-chunks per w1 DMA (1 MiB transfers)
N_WARM = 10  # scratch f32 matmuls (~1.1us each) to warm the PE clock gate

_compiled_cache = {}

# Set by the most recent kernel() call when BASS_KERNEL_TRACE=1: HW exec ns.
last_exec_time_ns = None
last_results = None


def _chunk_sizes(C):
    """Split C token columns into chunks <= 512, as evenly as possible.

    C >= 512 always (max expert load >= 4096/8), so chunks land in
    [256, 512] and float32r matmuls run at full 1 cycle/row speed.
    """
    nch = -(-C // 512)
    base, rem = divmod(C, nch)
    return [base + 1] * rem + [base] * (nch - rem)


def _w1_group_sizes():
    """m-chunks per w1 DMA group: two small leading groups so the first
    weight tile lands quickly at kernel start, then 1 MiB groups."""
    sizes = [1, 1]
    rest = MH - sum(sizes)
    sizes += [MG] * (rest // MG)
    return sizes


def _build_program(C):
    import concourse.mybir as mybir
    import concourse.tile as tile
    from concourse import bacc

    f32 = mybir.dt.float32
    f32r = mybir.dt.float32r

    nc = bacc.Bacc("TRN2", target_bir_lowering=False, debug=False,
                   num_devices=N_CORES)

    # Host-pretiled inputs (layouts chosen so each DMA is contiguous):
    #   xt  [128, KD*C] chunk-major: for token-chunk j (offset pf_j, len ch_j)
    #         xt[p, pf_j*KD + k*ch_j + c] = x[pf_j + c, k*128+p]
    #   w1  [128, MH*KD*128] grouped: group (m-offset o, size s) occupies
    #         cols [o*KD*P, (o+s)*KD*P); w1[p, off + (i*KD+k)*128+j] =
    #         W1[k*128+p, (o+i)*128+j]
    #   w2  [KD, 128, MH*128]       w2[d, p, k*128+j] = W2[k*128+p, d*128+j]
    #   b1t [128, MH]               b1t[p, m] = b1[m*128+p]
    # Output:
    #   yt  [KD, 128, C]            yt[d, p, c] = y[c, d*128+p]  (pre-b2)
    xt_d = nc.declare_dram_parameter("xt", [P, KD * C], f32r, isOutput=False)
    w1_d = nc.declare_dram_parameter("w1", [P, MH * KD * P], f32r, isOutput=False)
    w2_d = nc.declare_dram_parameter("w2", [KD, P, MH * P], f32r, isOutput=False)
    b1_d = nc.declare_dram_parameter("b1t", [P, MH], f32, isOutput=False)
    yt_d = nc.declare_dram_parameter("yt", [KD, P, C], f32, isOutput=True)

    chunks = _chunk_sizes(C)
    prefix = [0]
    for cn in chunks:
        prefix.append(prefix[-1] + cn)
    gsizes = _w1_group_sizes()

    with tile.TileContext(nc) as tc:
        with (
            tc.tile_pool(name="persist", bufs=1) as persist,
            tc.tile_pool(name="w1p", bufs=5) as w1p,
            tc.tile_pool(name="w2p", bufs=3) as w2p,
            tc.tile_pool(name="outp", bufs=4) as outp,
            tc.tile_pool(name="psum", bufs=6, space="PSUM") as psum,
        ):
            # --- PE warm-up: keep HAM at 8/8 through the input-load prologue
            scratch = persist.tile([P, 512], f32)
            nc.vector.memset(scratch[:], 0.0)
            warm_ps = psum.tile([P, 512], mybir.dt.float32, tag="warm", bufs=1)
            for _ in range(N_WARM):
                nc.tensor.matmul(warm_ps[:], scratch[:, :P], scratch[:],
                                 start=True, stop=True)

            # --- input loads: xt token-chunk blocks lead both HWDGE rings;
            # the first chunk (all the first matmul groups need) is the
            # smaller critical mass and rides the sync ring.
            xt = persist.tile([P, KD * C], f32r)
            split = prefix[1] * KD
            nc.sync.dma_start(out=xt[:, :split], in_=xt_d[:, :split])
            nc.scalar.dma_start(out=xt[:, split:], in_=xt_d[:, split:])
            b1t = persist.tile([P, MH], f32)
            nc.gpsimd.dma_start(out=b1t[:], in_=b1_d[:])
            ht = persist.tile([P, MH * C], f32r)

            # w2 tiles are prefetched on the scalar ring: the first bufs-many
            # late in phase A (so they don't contend with the w1 stream),
            # the rest at prefetch distance 3 inside phase B.
            w2_tiles = [None] * KD

            def load_w2(d):
                t = w2p.tile([P, MH * P], f32r, tag="w2d")
                nc.scalar.dma_start(out=t[:], in_=w2_d[d])
                w2_tiles[d] = t

            # ---- Phase A: hT = gelu(W1^T x^T + b1) ----
            m0 = 0
            for g, gs in enumerate(gsizes):
                w1g = w1p.tile([P, MG * KD * P], f32r, tag="w1g")
                nc.sync.dma_start(
                    out=w1g[:, :gs * KD * P],
                    in_=w1_d[:, m0 * KD * P:(m0 + gs) * KD * P])
                if g == len(gsizes) - 2:
                    load_w2(0)
                elif g == len(gsizes) - 1:
                    load_w2(1)
                for i in range(gs):
                    m = m0 + i
                    for j, cn in enumerate(chunks):
                        ps = psum.tile([P, 512], mybir.dt.float32, tag="ps")
                        xoff = prefix[j] * KD
                        for k in range(KD):
                            nc.tensor.matmul(
                                ps[:, :cn],
                                w1g[:, (i * KD + k) * P:(i * KD + k + 1) * P],
                                xt[:, xoff + k * cn:xoff + (k + 1) * cn],
                                start=(k == 0),
                                stop=(k == KD - 1),
                            )
                        nc.scalar.activation(
                            ht[:, m * C + prefix[j]:m * C + prefix[j] + cn],
                            ps[:, :cn],
                            mybir.ActivationFunctionType.Gelu,
                            bias=b1t[:, m:m + 1],
                        )
                m0 += gs

            # ---- Phase B: yT = W2^T hT ----
            for d in range(KD):
                w2d = w2_tiles[d]
                for j, cn in enumerate(chunks):
                    c0 = prefix[j]
                    ps = psum.tile([P, 512], mybir.dt.float32, tag="ps")
                    for k in range(MH):
                        nc.tensor.matmul(
                            ps[:, :cn],
                            w2d[:, k * P:(k + 1) * P],
                            ht[:, k * C + c0:k * C + c0 + cn],
                            start=(k == 0),
                            stop=(k == MH - 1),
                        )
                    ot = outp.tile([P, 512], f32, tag="ot")
                    nc.vector.tensor_copy(ot[:, :cn], ps[:, :cn])
                    nc.sync.dma_start(
                        out=yt_d[d, :, c0:c0 + cn], in_=ot[:, :cn]
                    )
                if d + 2 < KD:
                    load_w2(d + 2)

    nc.compile()
    return nc


def _get_program(C):
    if C not in _compiled_cache:
        _compiled_cache[C] = _build_program(C)
    return _compiled_cache[C]


def kernel(x, Wg, bg, W1, b1, W2, b2):
    global last_exec_time_ns, last_results
    from concourse.bass_utils import run_bass_kernel_spmd

    x = np.asarray(x, dtype=np.float32)
    Wg = np.asarray(Wg, dtype=np.float32)
    bg = np.asarray(bg, dtype=np.float32)
    W1 = np.asarray(W1, dtype=np.float32)
    b1 = np.asarray(b1, dtype=np.float32)
    W2 = np.asarray(W2, dtype=np.float32)
    b2 = np.asarray(b2, dtype=np.float32)

    B, S, D = x.shape
    T = B * S
    xf = x.reshape(T, D)

    # ---- Router (replicated gate, computed host-side as the dispatch step)
    logits = xf @ Wg + bg
    eidx = np.argmax(logits, axis=-1)

    tok = [np.nonzero(eidx == e)[0] for e in range(N_EXPERTS)]
    counts = [len(t) for t in tok]
    Cfull = max(max(counts), 512)
    Cfull = ((Cfull + 7) // 8) * 8  # mild alignment for DMA friendliness

    # SBUF fits a single pass up to ~C=704.  For pathologically imbalanced
    # routing, run multiple token blocks through the same program (weights
    # are re-sent per block; correctness over speed in that corner case).
    C_MAX = 704
    C = min(Cfull, C_MAX)
    n_blocks = -(-Cfull // C) if Cfull > C_MAX else 1
    if n_blocks > 1:
        C = min(C_MAX, ((-(-Cfull // n_blocks) + 7) // 8) * 8)

    nc = _get_program(C)
    chunks = _chunk_sizes(C)
    prefix = [0]
    for cn in chunks:
        prefix.append(prefix[-1] + cn)

    # ---- Per-core static operands (weights/biases, block-independent)
    w1s, w2s, b1s = [], [], []
    gsizes = _w1_group_sizes()
    for e in range(N_EXPERTS):
        # w1 flat [P, MH*KD*P]: per m-chunk block (i*KD+k)*P+j packing,
        # matching the device's per-group column slabs for any grouping.
        w1s.append(np.ascontiguousarray(
            W1[e].reshape(KD, P, MH, P)
            .transpose(2, 1, 0, 3)   # [m, p, k, j]
            .transpose(1, 0, 2, 3)   # [p, m, k, j]
            .reshape(P, MH * KD * P)
        ))
        w2s.append(np.ascontiguousarray(
            W2[e].reshape(MH, P, KD, P).transpose(2, 1, 0, 3).reshape(KD, P, MH * P)
        ))
        b1s.append(np.ascontiguousarray(b1[e].reshape(MH, P).T))

    trace = os.environ.get("BASS_KERNEL_TRACE", "") == "1"
    if trace:
        try:
            import axon_profile_shim

            axon_profile_shim.install()
        except ImportError:
            pass

    out = np.zeros((T, D), dtype=np.float32)
    for blk in range(n_blocks):
        in_maps = []
        blk_tok = []
        for e in range(N_EXPERTS):
            te = tok[e][blk * C:(blk + 1) * C]
            blk_tok.append(te)
            n_e = len(te)
            xe = xf[te]  # [n_e, D]
            # chunk-major xt: block j holds [P, KD*ch_j]
            xt = np.zeros((P, KD * C), dtype=np.float32)
            xeT = xe.T.reshape(KD, P, n_e)
            for j, cn in enumerate(chunks):
                lo = prefix[j]
                if lo >= n_e:
                    break
                hi = min(lo + cn, n_e)
                for k in range(KD):
                    xt[:, lo * KD + k * cn:lo * KD + k * cn + (hi - lo)] = \
                        xeT[k, :, lo:hi]
            in_maps.append({"xt": xt, "w1": w1s[e], "w2": w2s[e],
                            "b1t": b1s[e]})

        res = run_bass_kernel_spmd(nc, in_maps, list(range(N_CORES)),
                                   trace=trace)
        last_exec_time_ns = res.exec_time_ns
        last_results = res

        # ---- Combine: scatter tokens back, add b2 host-side
        for e in range(N_EXPERTS):
            te = blk_tok[e]
            n_e = len(te)
            if n_e == 0:
                continue
            yt = res.results[e]["yt"]  # [KD, P, C]
            ye = yt.reshape(D, C)[:, :n_e].T  # [n_e, D]
            out[te] = ye + b2[e][None, :]
    return out.reshape(B, S, D)


# revision 46
# speedup vs baseline: 1.0766x; 1.0074x over previous
"""MoE FFN (top-1 routing) Trainium2 kernel — expert-parallel across 8 cores.

Strategy (per the expert-parallel sharding hint): the router gate and the
token dispatch ARE the sharding step, performed on the host inside kernel():
  - host computes router logits (x @ Wg + bg) and argmax expert ids
  - tokens are gathered per expert, padded to capacity C = max expert load
  - core e receives expert e's W1/W2/b1 plus its routed tokens, pre-tiled
    into contiguous-DMA layouts
  - the device runs the full FFN (both matmuls + exact gelu) in float32r
  - host scatters per-expert outputs back (adds b2 there, it is per-token
    constant) and un-shards to the full [B, S, D] output

Device kernel per core (C tokens, D=1024, H=4096):
  phase A: hT[m*128+j, c] = gelu(sum_k W1tile[k,m].T x^T[k] + b1), m in 0..31
  phase B: yT[d*128+j, c] = sum_k W2tile[k,d].T hT[k],             d in 0..7
Both matmuls consume the weights in their natural [K, M] orientation as the
stationary operand, so no transposes are needed anywhere on the device.

DMA ring assignment: w1 stream on the sync HWDGE ring; xt + w2 stream on the
scalar HWDGE ring; output writes on gpsimd SWDGE.  A short burst of scratch
matmuls at kernel start keeps the PE HAM clock-gate warm through the input
load prologue.
"""

import os
import sys

import numpy as np

for _p in ("/opt/trn_rl_repo", "/root/.axon_site/_ro/trn_rl_repo"):
    if os.path.isdir(_p) and _p not in sys.path:
        sys.path.insert(0, _p)

D_MODEL = 1024
D_HIDDEN = 4096
N_EXPERTS = 8
N_CORES = 8
P = 128
KD = D_MODEL // P  # 8 k-chunks over d_model
MH = D_HIDDEN // P  # 32 m-chunks over d_hidden
MG = 2  # m-chunks per w1 DMA (1 MiB transfers)
N_WARM = 7  # scratch f32 matmuls (~1.1us each) to warm the PE clock gate

_compiled_cache = {}

# Set by the most recent kernel() call when BASS_KERNEL_TRACE=1: HW exec ns.
last_exec_time_ns = None
last_results = None


def _chunk_sizes(C):
    """Split C token columns into chunks <= 512, as evenly as possible.

    C >= 512 always (max expert load >= 4096/8), so chunks land in
    [256, 512] and float32r matmuls run at full 1 cycle/row speed.
    """
    nch = -(-C // 512)
    base, rem = divmod(C, nch)
    return [base + 1] * rem + [base] * (nch - rem)


def _w1_group_sizes():
    """m-chunks per w1 DMA group: two small leading groups so the first
    weight tile lands quickly at kernel start, then 1 MiB groups."""
    sizes = [1, 1]
    rest = MH - sum(sizes)
    sizes += [MG] * (rest // MG)
    return sizes


def _build_program(C):
    import concourse.mybir as mybir
    import concourse.tile as tile
    from concourse import bacc

    f32 = mybir.dt.float32
    f32r = mybir.dt.float32r

    nc = bacc.Bacc("TRN2", target_bir_lowering=False, debug=False,
                   num_devices=N_CORES)

    # Host-pretiled inputs (layouts chosen so each DMA is contiguous):
    #   xt  [128, KD*C] chunk-major: for token-chunk j (offset pf_j, len ch_j)
    #         xt[p, pf_j*KD + k*ch_j + c] = x[pf_j + c, k*128+p]
    #   w1  [128, MH*KD*128] grouped: group (m-offset o, size s) occupies
    #         cols [o*KD*P, (o+s)*KD*P); w1[p, off + (i*KD+k)*128+j] =
    #         W1[k*128+p, (o+i)*128+j]
    #   w2  [KD, 128, MH*128]       w2[d, p, k*128+j] = W2[k*128+p, d*128+j]
    #   b1t [128, MH]               b1t[p, m] = b1[m*128+p]
    # Output:
    #   yt  [KD, 128, C]            yt[d, p, c] = y[c, d*128+p]  (pre-b2)
    xt_d = nc.declare_dram_parameter("xt", [P, KD * C], f32r, isOutput=False)
    w1_d = nc.declare_dram_parameter("w1", [P, MH * KD * P], f32r, isOutput=False)
    w2_d = nc.declare_dram_parameter("w2", [KD, P, MH * P], f32r, isOutput=False)
    b1_d = nc.declare_dram_parameter("b1t", [P, MH], f32, isOutput=False)
    yt_d = nc.declare_dram_parameter("yt", [KD, P, C], f32, isOutput=True)

    chunks = _chunk_sizes(C)
    prefix = [0]
    for cn in chunks:
        prefix.append(prefix[-1] + cn)
    gsizes = _w1_group_sizes()

    with tile.TileContext(nc) as tc:
        with (
            tc.tile_pool(name="persist", bufs=1) as persist,
            tc.tile_pool(name="w1p", bufs=5) as w1p,
            tc.tile_pool(name="w2p", bufs=3) as w2p,
            tc.tile_pool(name="outp", bufs=4) as outp,
            tc.tile_pool(name="psum", bufs=6, space="PSUM") as psum,
        ):
            # --- PE warm-up: keep HAM at 8/8 through the input-load prologue
            scratch = persist.tile([P, 512], f32)
            nc.vector.memset(scratch[:], 0.0)
            warm_ps = psum.tile([P, 512], mybir.dt.float32, tag="warm", bufs=1)
            for _ in range(N_WARM):
                nc.tensor.matmul(warm_ps[:], scratch[:, :P], scratch[:],
                                 start=True, stop=True)

            # --- input loads: the whole startup critical mass rides the
            # sync ring in priority order (xt_c0, w1g0, xt_rest, w1g1, ...)
            # because the scalar ring's first data transfer is blocked ~3us
            # longer by the ACT table loads.  Scalar carries only w2.
            xt = persist.tile([P, KD * C], f32r)
            split = prefix[1] * KD
            nc.sync.dma_start(out=xt[:, :split], in_=xt_d[:, :split])
            b1t = persist.tile([P, MH], f32)
            nc.gpsimd.dma_start(out=b1t[:], in_=b1_d[:])
            ht = persist.tile([P, MH * C], f32r)

            # w2 tiles are prefetched on the scalar ring: the first bufs-many
            # late in phase A (so they don't contend with the w1 stream),
            # the rest at prefetch distance 3 inside phase B.
            w2_tiles = [None] * KD

            def load_w2(d):
                t = w2p.tile([P, MH * P], f32r, tag="w2d")
                nc.scalar.dma_start(out=t[:], in_=w2_d[d])
                w2_tiles[d] = t

            # ---- Phase A: hT = gelu(W1^T x^T + b1) ----
            m0 = 0
            for g, gs in enumerate(gsizes):
                w1g = w1p.tile([P, MG * KD * P], f32r, tag="w1g")
                nc.sync.dma_start(
                    out=w1g[:, :gs * KD * P],
                    in_=w1_d[:, m0 * KD * P:(m0 + gs) * KD * P])
                if g == 0:
                    nc.sync.dma_start(out=xt[:, split:], in_=xt_d[:, split:])
                if g == len(gsizes) - 2:
                    load_w2(0)
                elif g == len(gsizes) - 1:
                    load_w2(1)
                for i in range(gs):
                    m = m0 + i
                    for j, cn in enumerate(chunks):
                        ps = psum.tile([P, 512], mybir.dt.float32, tag="ps")
                        xoff = prefix[j] * KD
                        for k in range(KD):
                            nc.tensor.matmul(
                                ps[:, :cn],
                                w1g[:, (i * KD + k) * P:(i * KD + k + 1) * P],
                                xt[:, xoff + k * cn:xoff + (k + 1) * cn],
                                start=(k == 0),
                                stop=(k == KD - 1),
                            )
                        nc.scalar.activation(
                            ht[:, m * C + prefix[j]:m * C + prefix[j] + cn],
                            ps[:, :cn],
                            mybir.ActivationFunctionType.Gelu,
                            bias=b1t[:, m:m + 1],
                        )
                m0 += gs

            # ---- Phase B: yT = W2^T hT ----
            for d in range(KD):
                w2d = w2_tiles[d]
                for j, cn in enumerate(chunks):
                    c0 = prefix[j]
                    ps = psum.tile([P, 512], mybir.dt.float32, tag="ps")
                    for k in range(MH):
                        nc.tensor.matmul(
                            ps[:, :cn],
                            w2d[:, k * P:(k + 1) * P],
                            ht[:, k * C + c0:k * C + c0 + cn],
                            start=(k == 0),
                            stop=(k == MH - 1),
                        )
                    ot = outp.tile([P, 512], f32, tag="ot")
                    nc.vector.tensor_copy(ot[:, :cn], ps[:, :cn])
                    nc.sync.dma_start(
                        out=yt_d[d, :, c0:c0 + cn], in_=ot[:, :cn]
                    )
                if d + 2 < KD:
                    load_w2(d + 2)

    nc.compile()
    return nc


def _get_program(C):
    if C not in _compiled_cache:
        _compiled_cache[C] = _build_program(C)
    return _compiled_cache[C]


def kernel(x, Wg, bg, W1, b1, W2, b2):
    global last_exec_time_ns, last_results
    from concourse.bass_utils import run_bass_kernel_spmd

    x = np.asarray(x, dtype=np.float32)
    Wg = np.asarray(Wg, dtype=np.float32)
    bg = np.asarray(bg, dtype=np.float32)
    W1 = np.asarray(W1, dtype=np.float32)
    b1 = np.asarray(b1, dtype=np.float32)
    W2 = np.asarray(W2, dtype=np.float32)
    b2 = np.asarray(b2, dtype=np.float32)

    B, S, D = x.shape
    T = B * S
    xf = x.reshape(T, D)

    # ---- Router (replicated gate, computed host-side as the dispatch step)
    logits = xf @ Wg + bg
    eidx = np.argmax(logits, axis=-1)

    tok = [np.nonzero(eidx == e)[0] for e in range(N_EXPERTS)]
    counts = [len(t) for t in tok]
    Cfull = max(max(counts), 512)
    Cfull = ((Cfull + 7) // 8) * 8  # mild alignment for DMA friendliness

    # SBUF fits a single pass up to ~C=704.  For pathologically imbalanced
    # routing, run multiple token blocks through the same program (weights
    # are re-sent per block; correctness over speed in that corner case).
    C_MAX = 704
    C = min(Cfull, C_MAX)
    n_blocks = -(-Cfull // C) if Cfull > C_MAX else 1
    if n_blocks > 1:
        C = min(C_MAX, ((-(-Cfull // n_blocks) + 7) // 8) * 8)

    nc = _get_program(C)
    chunks = _chunk_sizes(C)
    prefix = [0]
    for cn in chunks:
        prefix.append(prefix[-1] + cn)

    # ---- Per-core static operands (weights/biases, block-independent)
    w1s, w2s, b1s = [], [], []
    gsizes = _w1_group_sizes()
    for e in range(N_EXPERTS):
        # w1 flat [P, MH*KD*P]: per m-chunk block (i*KD+k)*P+j packing,
        # matching the device's per-group column slabs for any grouping.
        w1s.append(np.ascontiguousarray(
            W1[e].reshape(KD, P, MH, P)
            .transpose(2, 1, 0, 3)   # [m, p, k, j]
            .transpose(1, 0, 2, 3)   # [p, m, k, j]
            .reshape(P, MH * KD * P)
        ))
        w2s.append(np.ascontiguousarray(
            W2[e].reshape(MH, P, KD, P).transpose(2, 1, 0, 3).reshape(KD, P, MH * P)
        ))
        b1s.append(np.ascontiguousarray(b1[e].reshape(MH, P).T))

    trace = os.environ.get("BASS_KERNEL_TRACE", "") == "1"
    if trace:
        try:
            import axon_profile_shim

            axon_profile_shim.install()
        except ImportError:
            pass

    out = np.zeros((T, D), dtype=np.float32)
    for blk in range(n_blocks):
        in_maps = []
        blk_tok = []
        for e in range(N_EXPERTS):
            te = tok[e][blk * C:(blk + 1) * C]
            blk_tok.append(te)
            n_e = len(te)
            xe = xf[te]  # [n_e, D]
            # chunk-major xt: block j holds [P, KD*ch_j]
            xt = np.zeros((P, KD * C), dtype=np.float32)
            xeT = xe.T.reshape(KD, P, n_e)
            for j, cn in enumerate(chunks):
                lo = prefix[j]
                if lo >= n_e:
                    break
                hi = min(lo + cn, n_e)
                for k in range(KD):
                    xt[:, lo * KD + k * cn:lo * KD + k * cn + (hi - lo)] = \
                        xeT[k, :, lo:hi]
            in_maps.append({"xt": xt, "w1": w1s[e], "w2": w2s[e],
                            "b1t": b1s[e]})

        res = run_bass_kernel_spmd(nc, in_maps, list(range(N_CORES)),
                                   trace=trace)
        last_exec_time_ns = res.exec_time_ns
        last_results = res

        # ---- Combine: scatter tokens back, add b2 host-side
        for e in range(N_EXPERTS):
            te = blk_tok[e]
            n_e = len(te)
            if n_e == 0:
                continue
            yt = res.results[e]["yt"]  # [KD, P, C]
            ye = yt.reshape(D, C)[:, :n_e].T  # [n_e, D]
            out[te] = ye + b2[e][None, :]
    return out.reshape(B, S, D)


# revision 47
# speedup vs baseline: 1.0923x; 1.0146x over previous
"""MoE FFN (top-1 routing) Trainium2 kernel — expert-parallel across 8 cores.

Strategy (per the expert-parallel sharding hint): the router gate and the
token dispatch ARE the sharding step, performed on the host inside kernel():
  - host computes router logits (x @ Wg + bg) and argmax expert ids
  - tokens are gathered per expert, padded to capacity C = max expert load
  - core e receives expert e's W1/W2/b1 plus its routed tokens, pre-tiled
    into contiguous-DMA layouts
  - the device runs the full FFN (both matmuls + exact gelu) in float32r
  - host scatters per-expert outputs back (adds b2 there, it is per-token
    constant) and un-shards to the full [B, S, D] output

Device kernel per core (C tokens, D=1024, H=4096):
  phase A: hT[m*128+j, c] = gelu(sum_k W1tile[k,m].T x^T[k] + b1), m in 0..31
  phase B: yT[d*128+j, c] = sum_k W2tile[k,d].T hT[k],             d in 0..7
Both matmuls consume the weights in their natural [K, M] orientation as the
stationary operand, so no transposes are needed anywhere on the device.

DMA ring assignment: w1 stream on the sync HWDGE ring; xt + w2 stream on the
scalar HWDGE ring; output writes on gpsimd SWDGE.  A short burst of scratch
matmuls at kernel start keeps the PE HAM clock-gate warm through the input
load prologue.
"""

import os
import sys

import numpy as np

for _p in ("/opt/trn_rl_repo", "/root/.axon_site/_ro/trn_rl_repo"):
    if os.path.isdir(_p) and _p not in sys.path:
        sys.path.insert(0, _p)

D_MODEL = 1024
D_HIDDEN = 4096
N_EXPERTS = 8
N_CORES = 8
P = 128
KD = D_MODEL // P  # 8 k-chunks over d_model
MH = D_HIDDEN // P  # 32 m-chunks over d_hidden
MG = 2  # m-chunks per w1 DMA (1 MiB transfers)
N_WARM = 7  # scratch f32 matmuls (~1.1us each) to warm the PE clock gate

_compiled_cache = {}

# Set by the most recent kernel() call when BASS_KERNEL_TRACE=1: HW exec ns.
last_exec_time_ns = None
last_results = None


def _chunk_sizes(C):
    """Split C token columns into chunks <= 512, as evenly as possible.

    C >= 512 always (max expert load >= 4096/8), so chunks land in
    [256, 512] and float32r matmuls run at full 1 cycle/row speed.
    """
    nch = -(-C // 512)
    base, rem = divmod(C, nch)
    return [base + 1] * rem + [base] * (nch - rem)


def _w1_group_sizes():
    """m-chunks per w1 DMA group: two small leading groups so the first
    weight tile lands quickly at kernel start, then 1 MiB groups."""
    sizes = [1, 1]
    rest = MH - sum(sizes)
    sizes += [MG] * (rest // MG)
    return sizes


def _build_program(C):
    import concourse.mybir as mybir
    import concourse.tile as tile
    from concourse import bacc

    f32 = mybir.dt.float32
    f32r = mybir.dt.float32r

    nc = bacc.Bacc("TRN2", target_bir_lowering=False, debug=False,
                   num_devices=N_CORES)

    # Host-pretiled inputs (layouts chosen so each DMA is contiguous):
    #   xt  [128, KD*C] chunk-major: for token-chunk j (offset pf_j, len ch_j)
    #         xt[p, pf_j*KD + k*ch_j + c] = x[pf_j + c, k*128+p]
    #   w1  [128, MH*KD*128] grouped: group (m-offset o, size s) occupies
    #         cols [o*KD*P, (o+s)*KD*P); w1[p, off + (i*KD+k)*128+j] =
    #         W1[k*128+p, (o+i)*128+j]
    #   w2  [KD, 128, MH*128]       w2[d, p, k*128+j] = W2[k*128+p, d*128+j]
    #   b1t [128, MH]               b1t[p, m] = b1[m*128+p]
    # Output:
    #   yt  [KD, 128, C]            yt[d, p, c] = y[c, d*128+p]  (pre-b2)
    xt_d = nc.declare_dram_parameter("xt", [P, KD * C], f32r, isOutput=False)
    w1_d = nc.declare_dram_parameter("w1", [P, MH * KD * P], f32r, isOutput=False)
    w2_d = nc.declare_dram_parameter("w2", [KD, P, MH * P], f32r, isOutput=False)
    b1_d = nc.declare_dram_parameter("b1t", [P, MH], f32, isOutput=False)
    yt_d = nc.declare_dram_parameter("yt", [KD, P, C], f32, isOutput=True)

    chunks = _chunk_sizes(C)
    prefix = [0]
    for cn in chunks:
        prefix.append(prefix[-1] + cn)
    gsizes = _w1_group_sizes()

    with tile.TileContext(nc) as tc:
        with (
            tc.tile_pool(name="persist", bufs=1) as persist,
            tc.tile_pool(name="w1p", bufs=5) as w1p,
            tc.tile_pool(name="w2p", bufs=3) as w2p,
            tc.tile_pool(name="outp", bufs=4) as outp,
            tc.tile_pool(name="psum", bufs=6, space="PSUM") as psum,
        ):
            # --- PE warm-up: keep HAM at 8/8 through the input-load prologue
            scratch = persist.tile([P, 512], f32)
            nc.vector.memset(scratch[:], 0.0)
            warm_ps = psum.tile([P, 512], mybir.dt.float32, tag="warm", bufs=1)
            for _ in range(N_WARM):
                nc.tensor.matmul(warm_ps[:], scratch[:, :P], scratch[:],
                                 start=True, stop=True)

            # --- input loads: the whole startup critical mass rides the
            # sync ring in priority order (xt_c0, w1g0, xt_rest, w1g1, ...)
            # because the scalar ring's first data transfer is blocked ~3us
            # longer by the ACT table loads.  Scalar carries only w2.
            xt = persist.tile([P, KD * C], f32r)
            split = prefix[1] * KD
            nc.sync.dma_start(out=xt[:, :split], in_=xt_d[:, :split])
            b1t = persist.tile([P, MH], f32)
            nc.gpsimd.dma_start(out=b1t[:], in_=b1_d[:])
            ht = persist.tile([P, MH * C], f32r)

            # w2 tiles are prefetched on the scalar ring: the first bufs-many
            # late in phase A (so they don't contend with the w1 stream),
            # the rest at prefetch distance 3 inside phase B.
            w2_tiles = [None] * KD

            def load_w2(d):
                t = w2p.tile([P, MH * P], f32r, tag="w2d")
                nc.scalar.dma_start(out=t[:], in_=w2_d[d])
                w2_tiles[d] = t

            # ---- Phase A: hT = gelu(W1^T x^T + b1) ----
            m0 = 0
            for g, gs in enumerate(gsizes):
                w1g = w1p.tile([P, MG * KD * P], f32r, tag="w1g")
                nc.sync.dma_start(
                    out=w1g[:, :gs * KD * P],
                    in_=w1_d[:, m0 * KD * P:(m0 + gs) * KD * P])
                if g == 0:
                    # second token chunk rides the scalar ring: it arrives
                    # ~15us (after the ACT table loads) which still beats a
                    # sync-FIFO slot behind w1g0, and keeps the w1 stream
                    # unobstructed on sync.
                    nc.scalar.dma_start(out=xt[:, split:], in_=xt_d[:, split:])
                if g == len(gsizes) - 2:
                    load_w2(0)
                elif g == len(gsizes) - 1:
                    load_w2(1)
                for i in range(gs):
                    m = m0 + i
                    for j, cn in enumerate(chunks):
                        ps = psum.tile([P, 512], mybir.dt.float32, tag="ps")
                        xoff = prefix[j] * KD
                        for k in range(KD):
                            nc.tensor.matmul(
                                ps[:, :cn],
                                w1g[:, (i * KD + k) * P:(i * KD + k + 1) * P],
                                xt[:, xoff + k * cn:xoff + (k + 1) * cn],
                                start=(k == 0),
                                stop=(k == KD - 1),
                            )
                        nc.scalar.activation(
                            ht[:, m * C + prefix[j]:m * C + prefix[j] + cn],
                            ps[:, :cn],
                            mybir.ActivationFunctionType.Gelu,
                            bias=b1t[:, m:m + 1],
                        )
                m0 += gs

            # ---- Phase B: yT = W2^T hT ----
            for d in range(KD):
                w2d = w2_tiles[d]
                for j, cn in enumerate(chunks):
                    c0 = prefix[j]
                    ps = psum.tile([P, 512], mybir.dt.float32, tag="ps")
                    for k in range(MH):
                        nc.tensor.matmul(
                            ps[:, :cn],
                            w2d[:, k * P:(k + 1) * P],
                            ht[:, k * C + c0:k * C + c0 + cn],
                            start=(k == 0),
                            stop=(k == MH - 1),
                        )
                    ot = outp.tile([P, 512], f32, tag="ot")
                    nc.vector.tensor_copy(ot[:, :cn], ps[:, :cn])
                    nc.sync.dma_start(
                        out=yt_d[d, :, c0:c0 + cn], in_=ot[:, :cn]
                    )
                if d + 2 < KD:
                    load_w2(d + 2)

    nc.compile()
    return nc


def _get_program(C):
    if C not in _compiled_cache:
        _compiled_cache[C] = _build_program(C)
    return _compiled_cache[C]


def kernel(x, Wg, bg, W1, b1, W2, b2):
    global last_exec_time_ns, last_results
    from concourse.bass_utils import run_bass_kernel_spmd

    x = np.asarray(x, dtype=np.float32)
    Wg = np.asarray(Wg, dtype=np.float32)
    bg = np.asarray(bg, dtype=np.float32)
    W1 = np.asarray(W1, dtype=np.float32)
    b1 = np.asarray(b1, dtype=np.float32)
    W2 = np.asarray(W2, dtype=np.float32)
    b2 = np.asarray(b2, dtype=np.float32)

    B, S, D = x.shape
    T = B * S
    xf = x.reshape(T, D)

    # ---- Router (replicated gate, computed host-side as the dispatch step)
    logits = xf @ Wg + bg
    eidx = np.argmax(logits, axis=-1)

    tok = [np.nonzero(eidx == e)[0] for e in range(N_EXPERTS)]
    counts = [len(t) for t in tok]
    Cfull = max(max(counts), 512)
    Cfull = ((Cfull + 7) // 8) * 8  # mild alignment for DMA friendliness

    # SBUF fits a single pass up to ~C=704.  For pathologically imbalanced
    # routing, run multiple token blocks through the same program (weights
    # are re-sent per block; correctness over speed in that corner case).
    C_MAX = 704
    C = min(Cfull, C_MAX)
    n_blocks = -(-Cfull // C) if Cfull > C_MAX else 1
    if n_blocks > 1:
        C = min(C_MAX, ((-(-Cfull // n_blocks) + 7) // 8) * 8)

    nc = _get_program(C)
    chunks = _chunk_sizes(C)
    prefix = [0]
    for cn in chunks:
        prefix.append(prefix[-1] + cn)

    # ---- Per-core static operands (weights/biases, block-independent)
    w1s, w2s, b1s = [], [], []
    gsizes = _w1_group_sizes()
    for e in range(N_EXPERTS):
        # w1 flat [P, MH*KD*P]: per m-chunk block (i*KD+k)*P+j packing,
        # matching the device's per-group column slabs for any grouping.
        w1s.append(np.ascontiguousarray(
            W1[e].reshape(KD, P, MH, P)
            .transpose(2, 1, 0, 3)   # [m, p, k, j]
            .transpose(1, 0, 2, 3)   # [p, m, k, j]
            .reshape(P, MH * KD * P)
        ))
        w2s.append(np.ascontiguousarray(
            W2[e].reshape(MH, P, KD, P).transpose(2, 1, 0, 3).reshape(KD, P, MH * P)
        ))
        b1s.append(np.ascontiguousarray(b1[e].reshape(MH, P).T))

    trace = os.environ.get("BASS_KERNEL_TRACE", "") == "1"
    if trace:
        try:
            import axon_profile_shim

            axon_profile_shim.install()
        except ImportError:
            pass

    out = np.zeros((T, D), dtype=np.float32)
    for blk in range(n_blocks):
        in_maps = []
        blk_tok = []
        for e in range(N_EXPERTS):
            te = tok[e][blk * C:(blk + 1) * C]
            blk_tok.append(te)
            n_e = len(te)
            xe = xf[te]  # [n_e, D]
            # chunk-major xt: block j holds [P, KD*ch_j]
            xt = np.zeros((P, KD * C), dtype=np.float32)
            xeT = xe.T.reshape(KD, P, n_e)
            for j, cn in enumerate(chunks):
                lo = prefix[j]
                if lo >= n_e:
                    break
                hi = min(lo + cn, n_e)
                for k in range(KD):
                    xt[:, lo * KD + k * cn:lo * KD + k * cn + (hi - lo)] = \
                        xeT[k, :, lo:hi]
            in_maps.append({"xt": xt, "w1": w1s[e], "w2": w2s[e],
                            "b1t": b1s[e]})

        res = run_bass_kernel_spmd(nc, in_maps, list(range(N_CORES)),
                                   trace=trace)
        last_exec_time_ns = res.exec_time_ns
        last_results = res

        # ---- Combine: scatter tokens back, add b2 host-side
        for e in range(N_EXPERTS):
            te = blk_tok[e]
            n_e = len(te)
            if n_e == 0:
                continue
            yt = res.results[e]["yt"]  # [KD, P, C]
            ye = yt.reshape(D, C)[:, :n_e].T  # [n_e, D]
            out[te] = ye + b2[e][None, :]
    return out.reshape(B, S, D)
